# revision 4
# baseline (speedup 1.0000x reference)
"""Trainium2 Bass kernel for nn_CrossAttention (B=4, Sx=4096, Sy=512, D=1024, H=8).

Sharding: 8 cores = (batch, query-half). Each core handles 2048 query rows of one
batch; K/V projections for that batch are computed locally (replicated across the
2 cores sharing a batch). The output projection is fully local, so no collectives
are needed; each core writes its own [2048, 1024] output slice.

Layouts are arranged so no on-device transposes are needed:
  qT[d, q]   = Wq.T @ xT        (xT pre-transposed on host)
  kT[d, s]   = (Wk/sqrt(dh)).T @ yT
  v[s, d]    = yT.T @ Wv
  scT[s, q]  = kT_h_chunk.T @ qT_h            (per head, Sy chunks of 128)
  eT         = exp(scT)                        (no max-subtract: |scores| ~ O(1))
  Z[h, q]    = onehot_h.T @ eT  (accumulated for all 8 heads into one [8,512] PSUM
               tile -> ONE batched reciprocal per window)
  aT[d, q]   = v_chunk.T @ eT, then aT *= (1/Z_h), broadcast across partitions
               via a K=8 row-selector matmul on the [8,512] reciprocal tile
  out[q, n]  = sum_h aT_h_chunk.T @ Wo_h + (bv @ Wo + bo)

v2 pipeline notes (vs the first working version):
  - exp is fused 1024-wide (2 activations/head instead of 4) so the scalar
    engine is no longer the rate limiter of the attention phase, and the
    Z/AV matmuls of head h are emitted after the scores of head h+1
    (1-head software pipeline) so they never wait on a fresh exp.
  - PSUM: sc pool 2x[128,2,512] (4 banks), shared qT/bcast/out-proj pool
    2x[128,512] (2), at 1, z 1  -> exactly 8 banks.
  - reciprocal -> reciprocal_approx_fast (~5x faster, 18 bits is plenty);
    PSUM->SBUF at-copies and the output bias add moved to the idle
    gpsimd(Pool) engine; DVE keeps only recip + zr cast + at normalize.
  - DMA: yt + small tensors on the gpsimd queue (its sequencer is ready
    ~0.7us before sync), wk split in 4 column pieces so the first kT matmul
    starts at ~8.5us instead of 17us; all weight loads issued up front from
    dedicated tiles in need-order (wk, wv, xw0, wq, wo).
All matmuls are bf16 (1 cycle/row on the PE).
"""
import sys
import types
import math
import numpy as np

sys.path.insert(0, "/opt/trn_rl_repo")

B, SX, SY, DE, DC, H, DH = 4, 4096, 512, 1024, 768, 8, 128
NCORES = 8
ROWS = B * SX // NCORES      # 2048 query rows per core
NW = ROWS // 512             # 4 windows of 512 rows
KQ = DE // 128               # 8 k-chunks for q/out projections
KY = DC // 128               # 6 k-chunks for k/v projections
SC = SY // 128               # 4 Sy chunks

MM_DTYPE = "bf16"            # "bf16" | "f32r"  (matmul operand precision)


def _install_shims():
    """antenv.axon_hooks is missing in this image; register the NTFF profile hook
    so trace=True works, and neuter the fish-bucket artifact upload."""
    if "antenv.axon_hooks" in sys.modules:
        return
    import antenv
    mod = types.ModuleType("antenv.axon_hooks")
    _h = [None]
    mod.set_axon_ntff_profile_hook = lambda h: _h.__setitem__(0, h)
    mod.get_axon_ntff_profile_hook = lambda: _h[0]
    sys.modules["antenv.axon_hooks"] = mod
    antenv.axon_hooks = mod
    try:
        from trn_agent_boot.trn_boot import _ntff_profile_via_ctypes
        mod.set_axon_ntff_profile_hook(
            _ntff_profile_via_ctypes("/opt/axon/libaxon_pjrt.so"))
    except Exception:
        pass
    from concourse import bass_utils
    bass_utils.upload_artifacts = lambda tmpdir: "local://" + tmpdir


_NC_CACHE = {}


def _build_nc(mm_dtype):
    from concourse import bacc, mybir
    from concourse.tile import TileContext

    F32 = mybir.dt.float32
    MMD = mybir.dt.bfloat16 if mm_dtype == "bf16" else mybir.dt.float32r
    Identity = mybir.ActivationFunctionType.Identity
    Exp = mybir.ActivationFunctionType.Exp
    ADD = mybir.AluOpType.add

    nc = bacc.Bacc(None, target_bir_lowering=False)
    xT = nc.dram_tensor("xT", [DE, ROWS], MMD, kind="ExternalInput")
    yT = nc.dram_tensor("yT", [DC, SY], MMD, kind="ExternalInput")
    wq = nc.dram_tensor("wq", [DE, DE], MMD, kind="ExternalInput")
    wk = nc.dram_tensor("wk", [DC, DE], MMD, kind="ExternalInput")
    wv = nc.dram_tensor("wv", [DC, DE], MMD, kind="ExternalInput")
    wo = nc.dram_tensor("wo", [DE, DE], MMD, kind="ExternalInput")
    bq_d = nc.dram_tensor("bq", [DE], F32, kind="ExternalInput")
    bk_d = nc.dram_tensor("bk", [DE], F32, kind="ExternalInput")
    bo_d = nc.dram_tensor("bo", [DE], F32, kind="ExternalInput")
    oh_d = nc.dram_tensor("oh", [128, H * H], MMD, kind="ExternalInput")
    sel_d = nc.dram_tensor("sel", [8, H * 128], MMD, kind="ExternalInput")
    out = nc.dram_tensor("out", [ROWS, DE], F32, kind="ExternalOutput")

    def chunked(ap, p=128):
        # DRAM [K, N] -> [p, K/p, N] view with the 128-partition dim first
        return ap.rearrange("(c p) n -> c p n", p=p).transpose([1, 0, 2])

    with TileContext(nc) as tc:
        with (
            tc.tile_pool(name="consts", bufs=1) as consts,
            tc.tile_pool(name="xwp", bufs=2) as xwp,
            tc.tile_pool(name="qtp", bufs=2) as qtp,
            tc.tile_pool(name="exp_p", bufs=3) as exp_p,
            tc.tile_pool(name="atp", bufs=2) as atp,
            tc.tile_pool(name="fop", bufs=3) as fop,
            tc.tile_pool(name="csp", bufs=2) as csp,
            tc.tile_pool(name="ps_m", bufs=2, space="PSUM") as ps_m,
            tc.tile_pool(name="ps_sc", bufs=2, space="PSUM") as ps_sc,
            tc.tile_pool(name="ps_at", bufs=1, space="PSUM") as ps_at,
            tc.tile_pool(name="ps_z", bufs=1, space="PSUM") as ps_z,
        ):
            # ---- resident constants ----
            wq_t = consts.tile([128, KQ, DE], MMD)
            wo_t = consts.tile([128, KQ, DE], MMD)
            kt = consts.tile([128, H, SY], MMD)       # kT: [d-part, head, Sy]
            vt = consts.tile([128, SC, DE], MMD)      # v: [Sy-part, Sy-chunk, d]
            bo_bc = consts.tile([128, DE], F32)
            bq_t = consts.tile([128, KQ], F32)
            bk_t = consts.tile([128, KQ], F32)
            oh_t = consts.tile([128, H, H], MMD)      # onehot: col h of slice [:,h,:]
            sel_t = consts.tile([8, H, 128], MMD)     # row-selector: row h of [:,h,:]
            yt = consts.tile([128, KY, SY], MMD)
            wkp = consts.tile([128, 4, KY, 256], MMD)  # wk in 4 column pieces
            wvp = consts.tile([128, 2, KY, 512], MMD)  # wv in 2 column halves

            # DMA issue order is execution order per queue. gpsimd's sequencer
            # wakes ~0.7us before sync, so the very first dependency (yt) goes
            # there; sync then streams everything else in PE need-order.
            nc.gpsimd.dma_start(out=yt[:], in_=chunked(yT[:]))
            nc.gpsimd.dma_start(out=bk_t[:], in_=bk_d[:].rearrange("(m p) -> p m", p=128))
            nc.gpsimd.dma_start(out=bq_t[:], in_=bq_d[:].rearrange("(m p) -> p m", p=128))
            nc.gpsimd.dma_start(out=oh_t[:], in_=oh_d[:].rearrange("p (h m) -> p h m", h=H))
            nc.gpsimd.dma_start(out=sel_t[:], in_=sel_d[:].rearrange("p (h m) -> p h m", h=H))
            nc.gpsimd.dma_start(out=bo_bc[:], in_=bo_d[:].partition_broadcast(128))
            for i in range(4):
                nc.sync.dma_start(out=wkp[:, i], in_=chunked(wk[:, i * 256:(i + 1) * 256]))
            for j in range(2):
                nc.sync.dma_start(out=wvp[:, j], in_=chunked(wv[:, j * 512:(j + 1) * 512]))
            xw0 = xwp.tile([128, KQ, 512], MMD, tag="xw")
            nc.sync.dma_start(out=xw0[:], in_=chunked(xT[:, 0:512]))
            nc.sync.dma_start(out=wq_t[:], in_=chunked(wq[:]))
            nc.sync.dma_start(out=wo_t[:], in_=chunked(wo[:]))

            # kT[d, s] = (Wk').T @ yT + bk'
            for m in range(8):
                ps = ps_m.tile([128, SY], F32, tag="a")
                for k in range(KY):
                    nc.tensor.matmul(ps[:], wkp[:, m // 2, k, (m % 2) * 128:(m % 2 + 1) * 128],
                                     yt[:, k, :], start=(k == 0), stop=(k == KY - 1))
                nc.scalar.activation(out=kt[:, m, :], in_=ps[:], func=Identity,
                                     bias=bk_t[:, m:m + 1], scale=1.0)
            # v[s, d] = yT.T @ Wv  (bv folded into bo_eff on host)
            for j in range(2):
                for sy in range(SC):
                    ps = ps_m.tile([128, SY], F32, tag="a")
                    for k in range(KY):
                        nc.tensor.matmul(ps[:], yt[:, k, sy * 128:(sy + 1) * 128],
                                         wvp[:, j, k, :], start=(k == 0), stop=(k == KY - 1))
                    nc.vector.tensor_copy(vt[:, sy, j * 512:(j + 1) * 512], ps[:])

            # ---- main loop over query windows of 512 rows ----
            # Window epilogue (normalize + out-proj) is deferred and emitted
            # after the NEXT window's qT matmuls, so the reciprocal/broadcast
            # chain overlaps PE work instead of stalling it.
            pending = None  # (w, at, z_ps) awaiting normalize + out-proj

            def emit_epilogue(p):
                pw, p_at, p_z = p
                zr = csp.tile([8, 512], F32, tag="zr")
                nc.vector.reciprocal_approx_fast(out=zr[:], in_=p_z[:])
                zr_m = csp.tile([8, 512], MMD, tag="zr_m")
                nc.vector.tensor_copy(zr_m[:], zr[:])
                for h in range(H):
                    bc_ps = ps_m.tile([128, 512], F32, tag="a")
                    nc.tensor.matmul(bc_ps[:], sel_t[:, h, :], zr_m[:],
                                     start=True, stop=True)
                    nc.vector.tensor_mul(p_at[:, h, :], p_at[:, h, :], bc_ps[:])
                # out[q, n] = sum_h aT_h.T @ Wo_h + bo_eff
                for qc in range(4):
                    for nh in range(2):
                        ps = ps_m.tile([128, 512], F32, tag="a")
                        for h in range(H):
                            nc.tensor.matmul(ps[:], p_at[:, h, qc * 128:(qc + 1) * 128],
                                             wo_t[:, h, nh * 512:(nh + 1) * 512],
                                             start=(h == 0), stop=(h == H - 1))
                        fo = fop.tile([128, 512], F32)
                        nc.vector.tensor_tensor(fo[:], ps[:],
                                                bo_bc[:, nh * 512:(nh + 1) * 512], ADD)
                        r0 = pw * 512 + qc * 128
                        nc.sync.dma_start(out=out[r0:r0 + 128, nh * 512:(nh + 1) * 512],
                                          in_=fo[:])

            for w in range(NW):
                if w == 0:
                    xw = xw0
                else:
                    xw = xwp.tile([128, KQ, 512], MMD, tag="xw")
                    nc.sync.dma_start(out=xw[:], in_=chunked(xT[:, w * 512:(w + 1) * 512]))

                # qT[d, q] = Wq.T @ xw + bq
                qt = qtp.tile([128, H, 512], MMD)
                for m in range(H):
                    ps = ps_m.tile([128, 512], F32, tag="a")
                    for k in range(KQ):
                        nc.tensor.matmul(ps[:], wq_t[:, k, m * 128:(m + 1) * 128],
                                         xw[:, k, :], start=(k == 0), stop=(k == KQ - 1))
                    nc.scalar.activation(out=qt[:, m, :], in_=ps[:], func=Identity,
                                         bias=bq_t[:, m:m + 1], scale=1.0)

                if pending is not None:
                    emit_epilogue(pending)

                at = atp.tile([128, H, 512], MMD)
                z_ps = ps_z.tile([8, 512], F32, tag="z")

                def flush(h, ex):
                    # softmax denominators for all heads -> one [8,512] PSUM tile
                    for kc in range(SC):
                        nc.tensor.matmul(z_ps[:], oh_t[:, h, :], ex[:, kc, :],
                                         start=(h == 0 and kc == 0),
                                         stop=(h == H - 1 and kc == SC - 1))
                    # aT[d, q] = v_h.T @ eT (normalized in the deferred epilogue)
                    at_ps = ps_at.tile([128, 512], F32, tag="at")
                    for kc in range(SC):
                        nc.tensor.matmul(at_ps[:], vt[:, kc, h * 128:(h + 1) * 128],
                                         ex[:, kc, :], start=(kc == 0),
                                         stop=(kc == SC - 1))
                    nc.vector.tensor_copy(at[:, h, :], at_ps[:])

                # scores + exp for head h; Z/AV of head h-1 emitted after, so
                # they never wait on a fresh exp (1-head software pipeline).
                prev = None
                for h in range(H):
                    pa = ps_sc.tile([128, 2, 512], F32, tag="sc")
                    pb = ps_sc.tile([128, 2, 512], F32, tag="sc")
                    ex = exp_p.tile([128, SC, 512], MMD)
                    for kc in range(SC):
                        phalf = pa if kc < 2 else pb
                        nc.tensor.matmul(phalf[:, kc % 2, :],
                                         kt[:, h, kc * 128:(kc + 1) * 128],
                                         qt[:, h, :], start=True, stop=True)
                    nc.scalar.activation(out=ex[:, 0:2, :], in_=pa[:], func=Exp)
                    nc.scalar.activation(out=ex[:, 2:4, :], in_=pb[:], func=Exp)
                    if prev is not None:
                        flush(*prev)
                    prev = (h, ex)
                flush(*prev)
                pending = (w, at, z_ps)

            emit_epilogue(pending)
    nc.finalize()
    return nc


def _to_mm(a, mm_dtype):
    if mm_dtype == "bf16":
        import ml_dtypes
        return np.ascontiguousarray(a).astype(ml_dtypes.bfloat16)
    return np.ascontiguousarray(a.astype(np.float32))


def _prep_inputs(mm_dtype, x, y, Wq, bq, Wk, bk, Wv, bv, Wo, bo):
    x = np.asarray(x, dtype=np.float32)
    y = np.asarray(y, dtype=np.float32).reshape(B, SY, DC)
    scale = 1.0 / math.sqrt(DH)
    Wq_m = _to_mm(np.asarray(Wq, np.float32), mm_dtype)
    wk_m = _to_mm(np.asarray(Wk, np.float32) * scale, mm_dtype)
    wv_m = _to_mm(np.asarray(Wv, np.float32), mm_dtype)
    wo_m = _to_mm(np.asarray(Wo, np.float32), mm_dtype)
    bk_s = np.asarray(bk, dtype=np.float32) * scale
    bo_eff = (np.asarray(bv, dtype=np.float64) @ np.asarray(Wo, dtype=np.float64)
              + np.asarray(bo, dtype=np.float64)).astype(np.float32)
    oh = np.zeros((128, H, H), np.float32)
    for h in range(H):
        oh[:, h, h] = 1.0
    oh = _to_mm(oh.reshape(128, H * H), mm_dtype)
    sel = np.zeros((8, H, 128), np.float32)
    for h in range(H):
        sel[h, h, :] = 1.0
    sel = _to_mm(sel.reshape(8, H * 128), mm_dtype)
    bq = np.asarray(bq, dtype=np.float32)

    in_maps = []
    for c in range(NCORES):
        b, hf = divmod(c, NCORES // B)
        xs = x[b, hf * ROWS:(hf + 1) * ROWS, :]
        in_maps.append({
            "xT": _to_mm(xs.T, mm_dtype),
            "yT": _to_mm(y[b].T, mm_dtype),
            "wq": Wq_m, "wk": wk_m, "wv": wv_m, "wo": wo_m,
            "bq": bq, "bk": bk_s, "bo": bo_eff, "oh": oh, "sel": sel,
        })
    return in_maps


def _run(inputs, trace=False, mm_dtype=None):
    _install_shims()
    from concourse.bass_utils import run_bass_kernel_spmd
    mm_dtype = mm_dtype or MM_DTYPE
    if mm_dtype not in _NC_CACHE:
        _NC_CACHE[mm_dtype] = _build_nc(mm_dtype)
    nc = _NC_CACHE[mm_dtype]
    in_maps = _prep_inputs(mm_dtype, **inputs)
    res = run_bass_kernel_spmd(nc, in_maps, list(range(NCORES)), trace=trace)
    outf = np.empty((B, SX, DE), dtype=np.float32)
    for c in range(NCORES):
        b, hf = divmod(c, NCORES // B)
        outf[b, hf * ROWS:(hf + 1) * ROWS, :] = res.results[c]["out"]
    return outf, res


def kernel(**inputs):
    out, _ = _run(inputs, trace=False)
    return out


# revision 5
# speedup vs baseline: 1.0214x; 1.0214x over previous
"""Trainium2 Bass kernel for nn_CrossAttention (B=4, Sx=4096, Sy=512, D=1024, H=8).

Sharding: 8 cores = (batch, query-half). Each core handles 2048 query rows of one
batch; K/V projections for that batch are computed locally (replicated across the
2 cores sharing a batch). The output projection is fully local, so no collectives
are needed; each core writes its own [2048, 1024] output slice.

Layouts are arranged so no on-device transposes are needed:
  qT[d, q]   = Wq.T @ xT        (xT pre-transposed on host)
  kT[d, s]   = (Wk/sqrt(dh)).T @ yT
  v[s, d]    = yT.T @ Wv
  scT[s, q]  = kT_h_chunk.T @ qT_h            (per head, Sy chunks of 128)
  eT         = exp(scT)                        (no max-subtract: |scores| ~ O(1))
  Z[h, q]    = onehot_h.T @ (eT_01 + eT_23)   (gpsimd pre-adds the 4 Sy-chunks
               pairwise so the PE only runs 2 Z-matmuls per head)
  aT[d, q]   = v_chunk.T @ eT, then aT *= (1/Z_h), broadcast across partitions
               via a K=8 row-selector matmul on the [8,512] reciprocal tile
  out[q, n]  = sum_h aT_h_chunk.T @ Wo_h + (bv @ Wo + bo)

v3 pipeline notes:
  - All DRAM inputs are HOST-PACKED partition-major ([128, ...] with large
    contiguous runs per partition) so DMAs move 3-16KB per descriptor instead
    of 0.5-2KB matrix rows (the v2 yt load took 8.6us for 0.75MB).
  - 8 warmup matmuls on a memset tile run during the initial DMA wait so the
    PE p-state is at 2.4GHz when real work starts.
  - exp is fused 1024-wide; scores of head h overlap AV of h-1 and Z of h-2
    (2-deep software pipeline; Z waits on the gpsimd pre-add).
  - PSUM: sc pool 2x[128,2,512] (4 banks), shared qT/bcast/out-proj pool
    2x[128,512] (2), at 1, z 1  -> exactly 8 banks.
  - reciprocal_approx_fast (~5x faster than reciprocal, 18 bits);
    at-copies / bias-adds on DVE (gpsimd = Pool cannot touch PSUM).
All matmuls are bf16 (1 cycle/row on the PE).
"""
import sys
import types
import math
import numpy as np

sys.path.insert(0, "/opt/trn_rl_repo")

B, SX, SY, DE, DC, H, DH = 4, 4096, 512, 1024, 768, 8, 128
NCORES = 8
ROWS = B * SX // NCORES      # 2048 query rows per core
NW = ROWS // 512             # 4 windows of 512 rows
KQ = DE // 128               # 8 k-chunks for q/out projections
KY = DC // 128               # 6 k-chunks for k/v projections
SC = SY // 128               # 4 Sy chunks

MM_DTYPE = "bf16"            # "bf16" | "f32r"  (matmul operand precision)


def _install_shims():
    """antenv.axon_hooks is missing in this image; register the NTFF profile hook
    so trace=True works, and neuter the fish-bucket artifact upload."""
    if "antenv.axon_hooks" in sys.modules:
        return
    import antenv
    mod = types.ModuleType("antenv.axon_hooks")
    _h = [None]
    mod.set_axon_ntff_profile_hook = lambda h: _h.__setitem__(0, h)
    mod.get_axon_ntff_profile_hook = lambda: _h[0]
    sys.modules["antenv.axon_hooks"] = mod
    antenv.axon_hooks = mod
    try:
        from trn_agent_boot.trn_boot import _ntff_profile_via_ctypes
        mod.set_axon_ntff_profile_hook(
            _ntff_profile_via_ctypes("/opt/axon/libaxon_pjrt.so"))
    except Exception:
        pass
    from concourse import bass_utils
    bass_utils.upload_artifacts = lambda tmpdir: "local://" + tmpdir


_NC_CACHE = {}


def _build_nc(mm_dtype):
    from concourse import bacc, mybir
    from concourse.tile import TileContext

    F32 = mybir.dt.float32
    MMD = mybir.dt.bfloat16 if mm_dtype == "bf16" else mybir.dt.float32r
    Identity = mybir.ActivationFunctionType.Identity
    Exp = mybir.ActivationFunctionType.Exp
    ADD = mybir.AluOpType.add

    nc = bacc.Bacc(None, target_bir_lowering=False)
    # all big inputs are host-packed partition-major (see _prep_inputs)
    xT = nc.dram_tensor("xT", [128, NW, KQ, 512], MMD, kind="ExternalInput")
    yT = nc.dram_tensor("yT", [128, KY, SY], MMD, kind="ExternalInput")
    wq = nc.dram_tensor("wq", [128, KQ, DE], MMD, kind="ExternalInput")
    wk = nc.dram_tensor("wk", [128, 4, KY, 256], MMD, kind="ExternalInput")
    wv = nc.dram_tensor("wv", [128, 2, KY, 512], MMD, kind="ExternalInput")
    wo = nc.dram_tensor("wo", [128, KQ, DE], MMD, kind="ExternalInput")
    bq_d = nc.dram_tensor("bq", [DE], F32, kind="ExternalInput")
    bk_d = nc.dram_tensor("bk", [DE], F32, kind="ExternalInput")
    bo_d = nc.dram_tensor("bo", [DE], F32, kind="ExternalInput")
    oh_d = nc.dram_tensor("oh", [128, H * H], MMD, kind="ExternalInput")
    sel_d = nc.dram_tensor("sel", [8, H * 128], MMD, kind="ExternalInput")
    out = nc.dram_tensor("out", [ROWS, DE], F32, kind="ExternalOutput")

    with TileContext(nc) as tc:
        with (
            tc.tile_pool(name="consts", bufs=1) as consts,
            tc.tile_pool(name="xwp", bufs=2) as xwp,
            tc.tile_pool(name="qtp", bufs=2) as qtp,
            tc.tile_pool(name="exp_p", bufs=3) as exp_p,
            tc.tile_pool(name="sump", bufs=3) as sump,
            tc.tile_pool(name="atp", bufs=2) as atp,
            tc.tile_pool(name="fop", bufs=3) as fop,
            tc.tile_pool(name="csp", bufs=2) as csp,
            tc.tile_pool(name="ps_m", bufs=2, space="PSUM") as ps_m,
            tc.tile_pool(name="ps_sc", bufs=2, space="PSUM") as ps_sc,
            tc.tile_pool(name="ps_at", bufs=1, space="PSUM") as ps_at,
            tc.tile_pool(name="ps_z", bufs=1, space="PSUM") as ps_z,
        ):
            # ---- resident constants ----
            wq_t = consts.tile([128, KQ, DE], MMD)
            wo_t = consts.tile([128, KQ, DE], MMD)
            kt = consts.tile([128, H, SY], MMD)       # kT: [d-part, head, Sy]
            vt = consts.tile([128, SC, DE], MMD)      # v: [Sy-part, Sy-chunk, d]
            bo_bc = consts.tile([128, DE], F32)
            bq_t = consts.tile([128, KQ], F32)
            bk_t = consts.tile([128, KQ], F32)
            oh_t = consts.tile([128, H, H], MMD)      # onehot: col h of slice [:,h,:]
            sel_t = consts.tile([8, H, 128], MMD)     # row-selector: row h of [:,h,:]
            yt = consts.tile([128, KY, SY], MMD)
            wkp = consts.tile([128, 4, KY, 256], MMD)  # wk in 4 column pieces
            wvp = consts.tile([128, 2, KY, 512], MMD)  # wv in 2 column halves
            wup = consts.tile([128, 512], MMD)         # warmup scratch

            # PE warmup: ~8 matmuls on a zeroed tile run during the initial DMA
            # wait so the p-state ramp completes before real work arrives.
            nc.vector.memset(wup[:], 0)
            for g in range(2):
                ps = ps_m.tile([128, 512], F32, tag="a")
                for r in range(4):
                    nc.tensor.matmul(ps[:], wup[:, 0:128], wup[:],
                                     start=(r == 0), stop=(r == 3))

            # DMA issue order is execution order per queue. gpsimd's sequencer
            # wakes ~0.7us before sync, so the very first dependency (yt) goes
            # there; sync then streams everything else in PE need-order.
            nc.gpsimd.dma_start(out=yt[:], in_=yT[:])
            nc.gpsimd.dma_start(out=bk_t[:], in_=bk_d[:].rearrange("(m p) -> p m", p=128))
            nc.gpsimd.dma_start(out=bq_t[:], in_=bq_d[:].rearrange("(m p) -> p m", p=128))
            nc.gpsimd.dma_start(out=oh_t[:], in_=oh_d[:].rearrange("p (h m) -> p h m", h=H))
            nc.gpsimd.dma_start(out=sel_t[:], in_=sel_d[:].rearrange("p (h m) -> p h m", h=H))
            nc.gpsimd.dma_start(out=bo_bc[:], in_=bo_d[:].partition_broadcast(128))
            for i in range(4):
                nc.sync.dma_start(out=wkp[:, i], in_=wk[:, i])
            for j in range(2):
                nc.sync.dma_start(out=wvp[:, j], in_=wv[:, j])
            xw0 = xwp.tile([128, KQ, 512], MMD, tag="xw")
            nc.sync.dma_start(out=xw0[:], in_=xT[:, 0])
            nc.sync.dma_start(out=wq_t[:], in_=wq[:])
            nc.sync.dma_start(out=wo_t[:], in_=wo[:])

            # kT[d, s] = (Wk').T @ yT + bk'
            for m in range(8):
                ps = ps_m.tile([128, SY], F32, tag="a")
                for k in range(KY):
                    nc.tensor.matmul(ps[:], wkp[:, m // 2, k, (m % 2) * 128:(m % 2 + 1) * 128],
                                     yt[:, k, :], start=(k == 0), stop=(k == KY - 1))
                nc.scalar.activation(out=kt[:, m, :], in_=ps[:], func=Identity,
                                     bias=bk_t[:, m:m + 1], scale=1.0)
            # v[s, d] = yT.T @ Wv  (bv folded into bo_eff on host)
            for j in range(2):
                for sy in range(SC):
                    ps = ps_m.tile([128, SY], F32, tag="a")
                    for k in range(KY):
                        nc.tensor.matmul(ps[:], yt[:, k, sy * 128:(sy + 1) * 128],
                                         wvp[:, j, k, :], start=(k == 0), stop=(k == KY - 1))
                    nc.vector.tensor_copy(vt[:, sy, j * 512:(j + 1) * 512], ps[:])

            # ---- main loop over query windows of 512 rows ----
            # Window epilogue (normalize + out-proj) is deferred and emitted
            # after the NEXT window's qT matmuls, so the reciprocal/broadcast
            # chain overlaps PE work instead of stalling it.
            pending = None  # (w, at, z_ps) awaiting normalize + out-proj

            def emit_epilogue(p):
                pw, p_at, p_z = p
                zr = csp.tile([8, 512], F32, tag="zr")
                nc.vector.reciprocal_approx_fast(out=zr[:], in_=p_z[:])
                zr_m = csp.tile([8, 512], MMD, tag="zr_m")
                nc.vector.tensor_copy(zr_m[:], zr[:])
                for h in range(H):
                    bc_ps = ps_m.tile([128, 512], F32, tag="a")
                    nc.tensor.matmul(bc_ps[:], sel_t[:, h, :], zr_m[:],
                                     start=True, stop=True)
                    nc.vector.tensor_mul(p_at[:, h, :], p_at[:, h, :], bc_ps[:])
                # out[q, n] = sum_h aT_h.T @ Wo_h + bo_eff
                for qc in range(4):
                    for nh in range(2):
                        ps = ps_m.tile([128, 512], F32, tag="a")
                        for h in range(H):
                            nc.tensor.matmul(ps[:], p_at[:, h, qc * 128:(qc + 1) * 128],
                                             wo_t[:, h, nh * 512:(nh + 1) * 512],
                                             start=(h == 0), stop=(h == H - 1))
                        fo = fop.tile([128, 512], F32)
                        nc.vector.tensor_tensor(fo[:], ps[:],
                                                bo_bc[:, nh * 512:(nh + 1) * 512], ADD)
                        r0 = pw * 512 + qc * 128
                        nc.sync.dma_start(out=out[r0:r0 + 128, nh * 512:(nh + 1) * 512],
                                          in_=fo[:])

            for w in range(NW):
                if w == 0:
                    xw = xw0
                else:
                    xw = xwp.tile([128, KQ, 512], MMD, tag="xw")
                    nc.sync.dma_start(out=xw[:], in_=xT[:, w])

                # qT[d, q] = Wq.T @ xw + bq
                qt = qtp.tile([128, H, 512], MMD)
                for m in range(H):
                    ps = ps_m.tile([128, 512], F32, tag="a")
                    for k in range(KQ):
                        nc.tensor.matmul(ps[:], wq_t[:, k, m * 128:(m + 1) * 128],
                                         xw[:, k, :], start=(k == 0), stop=(k == KQ - 1))
                    nc.scalar.activation(out=qt[:, m, :], in_=ps[:], func=Identity,
                                         bias=bq_t[:, m:m + 1], scale=1.0)

                if pending is not None:
                    emit_epilogue(pending)

                at = atp.tile([128, H, 512], MMD)
                z_ps = ps_z.tile([8, 512], F32, tag="z")

                def emit_av(h, ex):
                    # aT[d, q] = v_h.T @ eT (normalized in the deferred epilogue)
                    at_ps = ps_at.tile([128, 512], F32, tag="at")
                    for kc in range(SC):
                        nc.tensor.matmul(at_ps[:], vt[:, kc, h * 128:(h + 1) * 128],
                                         ex[:, kc, :], start=(kc == 0),
                                         stop=(kc == SC - 1))
                    nc.vector.tensor_copy(at[:, h, :], at_ps[:])

                def emit_z(h, es):
                    # softmax denominators for all heads -> one [8,512] PSUM tile
                    for j in range(2):
                        nc.tensor.matmul(z_ps[:], oh_t[:, h, :], es[:, j, :],
                                         start=(h == 0 and j == 0),
                                         stop=(h == H - 1 and j == 1))

                # 2-deep software pipeline: scores+exp for head h, AV of head
                # h-1, Z of head h-2 (Z waits on the gpsimd pair-sum of exps).
                hist = []  # [(h, ex, es)] most recent last
                for h in range(H):
                    pa = ps_sc.tile([128, 2, 512], F32, tag="sc")
                    pb = ps_sc.tile([128, 2, 512], F32, tag="sc")
                    ex = exp_p.tile([128, SC, 512], MMD)
                    for kc in range(SC):
                        phalf = pa if kc < 2 else pb
                        nc.tensor.matmul(phalf[:, kc % 2, :],
                                         kt[:, h, kc * 128:(kc + 1) * 128],
                                         qt[:, h, :], start=True, stop=True)
                    nc.scalar.activation(out=ex[:, 0:2, :], in_=pa[:], func=Exp)
                    nc.scalar.activation(out=ex[:, 2:4, :], in_=pb[:], func=Exp)
                    es = sump.tile([128, 2, 512], MMD)
                    nc.gpsimd.tensor_tensor(es[:], ex[:, 0:2, :], ex[:, 2:4, :], ADD)
                    if hist:
                        emit_av(hist[-1][0], hist[-1][1])
                    if len(hist) >= 2:
                        emit_z(hist[-2][0], hist[-2][2])
                    hist.append((h, ex, es))
                emit_av(hist[-1][0], hist[-1][1])
                emit_z(hist[-2][0], hist[-2][2])
                emit_z(hist[-1][0], hist[-1][2])
                pending = (w, at, z_ps)

            emit_epilogue(pending)
    nc.finalize()
    return nc


def _to_mm(a, mm_dtype):
    if mm_dtype == "bf16":
        import ml_dtypes
        return np.ascontiguousarray(a).astype(ml_dtypes.bfloat16)
    return np.ascontiguousarray(a.astype(np.float32))


def _pack(a, np_, npiece=None):
    """[K, N] -> [128, K//128, N] partition-major (optionally split N into
    npiece contiguous column pieces first: -> [128, npiece, K//128, N//npiece])."""
    K, N = a.shape
    if npiece:
        w = N // npiece
        return np.ascontiguousarray(
            a.reshape(K // 128, 128, npiece, w).transpose(1, 2, 0, 3))
    return np.ascontiguousarray(a.reshape(K // 128, 128, N).transpose(1, 0, 2))


def _prep_inputs(mm_dtype, x, y, Wq, bq, Wk, bk, Wv, bv, Wo, bo):
    x = np.asarray(x, dtype=np.float32)
    y = np.asarray(y, dtype=np.float32).reshape(B, SY, DC)
    scale = 1.0 / math.sqrt(DH)
    Wq_m = _to_mm(_pack(np.asarray(Wq, np.float32), np), mm_dtype)
    wk_m = _to_mm(_pack(np.asarray(Wk, np.float32) * scale, np, npiece=4), mm_dtype)
    wv_m = _to_mm(_pack(np.asarray(Wv, np.float32), np, npiece=2), mm_dtype)
    wo_m = _to_mm(_pack(np.asarray(Wo, np.float32), np), mm_dtype)
    bk_s = np.asarray(bk, dtype=np.float32) * scale
    bo_eff = (np.asarray(bv, dtype=np.float64) @ np.asarray(Wo, dtype=np.float64)
              + np.asarray(bo, dtype=np.float64)).astype(np.float32)
    oh = np.zeros((128, H, H), np.float32)
    for h in range(H):
        oh[:, h, h] = 1.0
    oh = _to_mm(oh.reshape(128, H * H), mm_dtype)
    sel = np.zeros((8, H, 128), np.float32)
    for h in range(H):
        sel[h, h, :] = 1.0
    sel = _to_mm(sel.reshape(8, H * 128), mm_dtype)
    bq = np.asarray(bq, dtype=np.float32)

    in_maps = []
    for c in range(NCORES):
        b, hf = divmod(c, NCORES // B)
        xs = x[b, hf * ROWS:(hf + 1) * ROWS, :]
        # xT [1024, 2048] -> [128, NW, KQ, 512]: window-major pieces so each
        # per-window DMA reads 8KB contiguous per partition.
        xtp = xs.T.reshape(KQ, 128, NW, 512).transpose(1, 2, 0, 3)
        in_maps.append({
            "xT": _to_mm(np.ascontiguousarray(xtp), mm_dtype),
            "yT": _to_mm(_pack(y[b].T, np), mm_dtype),
            "wq": Wq_m, "wk": wk_m, "wv": wv_m, "wo": wo_m,
            "bq": bq, "bk": bk_s, "bo": bo_eff, "oh": oh, "sel": sel,
        })
    return in_maps


def _run(inputs, trace=False, mm_dtype=None):
    _install_shims()
    from concourse.bass_utils import run_bass_kernel_spmd
    mm_dtype = mm_dtype or MM_DTYPE
    if mm_dtype not in _NC_CACHE:
        _NC_CACHE[mm_dtype] = _build_nc(mm_dtype)
    nc = _NC_CACHE[mm_dtype]
    in_maps = _prep_inputs(mm_dtype, **inputs)
    res = run_bass_kernel_spmd(nc, in_maps, list(range(NCORES)), trace=trace)
    outf = np.empty((B, SX, DE), dtype=np.float32)
    for c in range(NCORES):
        b, hf = divmod(c, NCORES // B)
        outf[b, hf * ROWS:(hf + 1) * ROWS, :] = res.results[c]["out"]
    return outf, res


def kernel(**inputs):
    out, _ = _run(inputs, trace=False)
    return out


# revision 8
# speedup vs baseline: 1.0444x; 1.0226x over previous
"""Trainium2 Bass kernel for nn_CrossAttention (B=4, Sx=4096, Sy=512, D=1024, H=8).

Sharding: 8 cores = (batch, query-half). Each core handles 2048 query rows of one
batch; K/V projections for that batch are computed locally (replicated across the
2 cores sharing a batch). The output projection is fully local, so no collectives
are needed; each core writes its own [2048, 1024] output slice.

Layouts are arranged so no on-device transposes are needed:
  qT[d, q]   = Wq.T @ xT        (xT pre-transposed on host)
  kT[d, s]   = (Wk/sqrt(dh)).T @ yT
  v[s, d]    = yT.T @ Wv
  scT[s, q]  = kT_h_chunk.T @ qT_h            (per head, Sy chunks of 128)
  eT         = exp(scT)                        (no max-subtract: |scores| ~ O(1))
  Z[h, q]    = onehot_h.T @ (eT_01 + eT_23)   (gpsimd pre-adds the 4 Sy-chunks
               pairwise so the PE only runs 2 Z-matmuls per head)
  aT[d, q]   = v_chunk.T @ eT, then aT *= (1/Z_h), broadcast across partitions
               via a K=8 row-selector matmul on the [8,512] reciprocal tile
  out[q, n]  = sum_h aT_h_chunk.T @ Wo_h + (bv @ Wo + bo)

v3 pipeline notes:
  - All DRAM inputs are HOST-PACKED partition-major ([128, ...] with large
    contiguous runs per partition) so DMAs move 3-16KB per descriptor instead
    of 0.5-2KB matrix rows (the v2 yt load took 8.6us for 0.75MB).
  - 8 warmup matmuls on a memset tile run during the initial DMA wait so the
    PE p-state is at 2.4GHz when real work starts.
  - exp is fused 1024-wide; scores of head h overlap AV of h-1 and Z of h-2
    (2-deep software pipeline; Z waits on the gpsimd pre-add).
  - PSUM: sc pool 2x[128,2,512] (4 banks), shared qT/bcast/out-proj pool
    2x[128,512] (2), at 1, z 1  -> exactly 8 banks.
  - reciprocal_approx_fast (~5x faster than reciprocal, 18 bits);
    at-copies / bias-adds on DVE (gpsimd = Pool cannot touch PSUM).
All matmuls are bf16 (1 cycle/row on the PE).
"""
import sys
import types
import math
import numpy as np

sys.path.insert(0, "/opt/trn_rl_repo")

B, SX, SY, DE, DC, H, DH = 4, 4096, 512, 1024, 768, 8, 128
NCORES = 8
ROWS = B * SX // NCORES      # 2048 query rows per core
NW = ROWS // 512             # 4 windows of 512 rows
KQ = DE // 128               # 8 k-chunks for q/out projections
KY = DC // 128               # 6 k-chunks for k/v projections
SC = SY // 128               # 4 Sy chunks

MM_DTYPE = "bf16"            # "bf16" | "f32r"  (matmul operand precision)


def _install_shims():
    """antenv.axon_hooks is missing in this image; register the NTFF profile hook
    so trace=True works, and neuter the fish-bucket artifact upload."""
    if "antenv.axon_hooks" in sys.modules:
        return
    import antenv
    mod = types.ModuleType("antenv.axon_hooks")
    _h = [None]
    mod.set_axon_ntff_profile_hook = lambda h: _h.__setitem__(0, h)
    mod.get_axon_ntff_profile_hook = lambda: _h[0]
    sys.modules["antenv.axon_hooks"] = mod
    antenv.axon_hooks = mod
    try:
        from trn_agent_boot.trn_boot import _ntff_profile_via_ctypes
        mod.set_axon_ntff_profile_hook(
            _ntff_profile_via_ctypes("/opt/axon/libaxon_pjrt.so"))
    except Exception:
        pass
    from concourse import bass_utils
    bass_utils.upload_artifacts = lambda tmpdir: "local://" + tmpdir


_NC_CACHE = {}


def _build_nc(mm_dtype):
    from concourse import bacc, mybir
    from concourse.tile import TileContext

    F32 = mybir.dt.float32
    MMD = mybir.dt.bfloat16 if mm_dtype == "bf16" else mybir.dt.float32r
    Identity = mybir.ActivationFunctionType.Identity
    Exp = mybir.ActivationFunctionType.Exp
    ADD = mybir.AluOpType.add

    nc = bacc.Bacc(None, target_bir_lowering=False)
    # all big inputs are host-packed partition-major (see _prep_inputs)
    xT = nc.dram_tensor("xT", [128, NW, KQ, 512], MMD, kind="ExternalInput")
    yT = nc.dram_tensor("yT", [128, KY, SY], MMD, kind="ExternalInput")
    wq = nc.dram_tensor("wq", [128, KQ, DE], MMD, kind="ExternalInput")
    wk = nc.dram_tensor("wk", [128, 4, KY, 256], MMD, kind="ExternalInput")
    wv = nc.dram_tensor("wv", [128, 2, KY, 512], MMD, kind="ExternalInput")
    wo = nc.dram_tensor("wo", [128, KQ, DE], MMD, kind="ExternalInput")
    bq_d = nc.dram_tensor("bq", [DE], F32, kind="ExternalInput")
    bk_d = nc.dram_tensor("bk", [DE], F32, kind="ExternalInput")
    bo_d = nc.dram_tensor("bo", [DE], F32, kind="ExternalInput")
    oh_d = nc.dram_tensor("oh", [128, H * H], MMD, kind="ExternalInput")
    sel_d = nc.dram_tensor("sel", [8, H * 128], MMD, kind="ExternalInput")
    out = nc.dram_tensor("out", [ROWS, DE], F32, kind="ExternalOutput")

    with TileContext(nc) as tc:
        with (
            tc.tile_pool(name="consts", bufs=1) as consts,
            tc.tile_pool(name="xwp", bufs=2) as xwp,
            tc.tile_pool(name="qtp", bufs=2) as qtp,
            tc.tile_pool(name="exp_p", bufs=3) as exp_p,
            tc.tile_pool(name="sump", bufs=3) as sump,
            tc.tile_pool(name="atp", bufs=2) as atp,
            tc.tile_pool(name="fop", bufs=3) as fop,
            tc.tile_pool(name="csp", bufs=2) as csp,
            tc.tile_pool(name="ps_m", bufs=2, space="PSUM") as ps_m,
            tc.tile_pool(name="ps_sc", bufs=2, space="PSUM") as ps_sc,
            tc.tile_pool(name="ps_at", bufs=1, space="PSUM") as ps_at,
            tc.tile_pool(name="ps_z", bufs=1, space="PSUM") as ps_z,
        ):
            # ---- resident constants ----
            wq_t = consts.tile([128, KQ, DE], MMD)
            wo_t = consts.tile([128, KQ, DE], MMD)
            kt = consts.tile([128, H, SY], MMD)       # kT: [d-part, head, Sy]
            vt = consts.tile([128, SC, DE], MMD)      # v: [Sy-part, Sy-chunk, d]
            bo_bc = consts.tile([128, DE], F32)
            bq_t = consts.tile([128, KQ], F32)
            bk_t = consts.tile([128, KQ], F32)
            oh_t = consts.tile([128, H, H], MMD)      # onehot: col h of slice [:,h,:]
            sel_t = consts.tile([8, H, 128], MMD)     # row-selector: row h of [:,h,:]
            yt = consts.tile([128, KY, SY], MMD)
            wkp = consts.tile([128, 4, KY, 256], MMD)  # wk in 4 column pieces
            wvp = consts.tile([128, 2, KY, 512], MMD)  # wv in 2 column halves
            wup = consts.tile([128, 512], MMD)         # warmup scratch

            # PE warmup: ~14 matmuls on a zeroed tile run during the initial DMA
            # wait so the p-state ramp completes before real work arrives.
            nc.vector.memset(wup[:], 0)
            for g in range(2):
                ps = ps_m.tile([128, 512], F32, tag="a")
                for r in range(7):
                    nc.tensor.matmul(ps[:], wup[:, 0:128], wup[:],
                                     start=(r == 0), stop=(r == 6))

            # DMA issue order is execution order per queue. The sync queue is
            # hardware-DGE (fast); gpsimd's is software-DGE (slow) and only
            # carries the tiny bias/selector tensors.
            nc.sync.dma_start(out=yt[:], in_=yT[:])
            nc.gpsimd.dma_start(out=bk_t[:], in_=bk_d[:].rearrange("(m p) -> p m", p=128))
            nc.gpsimd.dma_start(out=bq_t[:], in_=bq_d[:].rearrange("(m p) -> p m", p=128))
            nc.gpsimd.dma_start(out=oh_t[:], in_=oh_d[:].rearrange("p (h m) -> p h m", h=H))
            nc.gpsimd.dma_start(out=sel_t[:], in_=sel_d[:].rearrange("p (h m) -> p h m", h=H))
            nc.gpsimd.dma_start(out=bo_bc[:], in_=bo_d[:].partition_broadcast(128))
            for i in range(4):
                nc.sync.dma_start(out=wkp[:, i], in_=wk[:, i])
            for j in range(2):
                nc.sync.dma_start(out=wvp[:, j], in_=wv[:, j])
            xw0 = xwp.tile([128, KQ, 512], MMD, tag="xw")
            nc.sync.dma_start(out=xw0[:], in_=xT[:, 0])
            nc.sync.dma_start(out=wq_t[:], in_=wq[:])
            nc.sync.dma_start(out=wo_t[:], in_=wo[:])

            # kT[d, s] = (Wk').T @ yT + bk'
            for m in range(8):
                ps = ps_m.tile([128, SY], F32, tag="a")
                for k in range(KY):
                    nc.tensor.matmul(ps[:], wkp[:, m // 2, k, (m % 2) * 128:(m % 2 + 1) * 128],
                                     yt[:, k, :], start=(k == 0), stop=(k == KY - 1))
                nc.scalar.activation(out=kt[:, m, :], in_=ps[:], func=Identity,
                                     bias=bk_t[:, m:m + 1], scale=1.0)
            # v[s, d] = yT.T @ Wv  (bv folded into bo_eff on host)
            for j in range(2):
                for sy in range(SC):
                    ps = ps_m.tile([128, SY], F32, tag="a")
                    for k in range(KY):
                        nc.tensor.matmul(ps[:], yt[:, k, sy * 128:(sy + 1) * 128],
                                         wvp[:, j, k, :], start=(k == 0), stop=(k == KY - 1))
                    nc.vector.tensor_copy(vt[:, sy, j * 512:(j + 1) * 512], ps[:])

            # ---- main loop over query windows of 512 rows ----
            # Window epilogue (normalize + out-proj) is deferred and emitted
            # after the NEXT window's qT matmuls, so the reciprocal/broadcast
            # chain overlaps PE work instead of stalling it.
            pending = None  # (w, at, z_ps) awaiting normalize + out-proj

            def emit_epilogue(p):
                pw, p_at, p_z = p
                zr = csp.tile([8, 512], F32, tag="zr")
                nc.vector.reciprocal_approx_fast(out=zr[:], in_=p_z[:])
                zr_m = csp.tile([8, 512], MMD, tag="zr_m")
                nc.vector.tensor_copy(zr_m[:], zr[:])
                # 6-deep broadcast ring (sc-pool halves + a-pool slots) so the
                # bc matmuls stream back-to-back instead of serializing at the
                # DVE multiply rate.
                bc_ap = [None] * H
                t1 = ps_sc.tile([128, 2, 512], F32, tag="sc")
                bc_ap[0], bc_ap[1] = t1[:, 0, :], t1[:, 1, :]
                t2 = ps_sc.tile([128, 2, 512], F32, tag="sc")
                bc_ap[2], bc_ap[3] = t2[:, 0, :], t2[:, 1, :]
                a1 = ps_m.tile([128, 512], F32, tag="a")
                a2 = ps_m.tile([128, 512], F32, tag="a")
                bc_ap[4], bc_ap[5] = a1[:], a2[:]
                t3 = ps_sc.tile([128, 2, 512], F32, tag="sc")
                bc_ap[6], bc_ap[7] = t3[:, 0, :], t3[:, 1, :]
                for h in range(H):
                    nc.tensor.matmul(bc_ap[h], sel_t[:, h, :], zr_m[:],
                                     start=True, stop=True)
                    nc.vector.tensor_mul(p_at[:, h, :], p_at[:, h, :], bc_ap[h])
                # out[q, n] = sum_h aT_h.T @ Wo_h + bo_eff. Two accumulation
                # groups open at a time, h-major, so each h-block only waits on
                # one normalize multiply and the PE never drains.
                for qc in range(4):
                    psA = ps_m.tile([128, 512], F32, tag="a")
                    psB = ps_m.tile([128, 512], F32, tag="a")
                    for h in range(H):
                        nc.tensor.matmul(psA[:], p_at[:, h, qc * 128:(qc + 1) * 128],
                                         wo_t[:, h, 0:512],
                                         start=(h == 0), stop=(h == H - 1))
                        nc.tensor.matmul(psB[:], p_at[:, h, qc * 128:(qc + 1) * 128],
                                         wo_t[:, h, 512:1024],
                                         start=(h == 0), stop=(h == H - 1))
                    r0 = pw * 512 + qc * 128
                    for nh, ps in ((0, psA), (1, psB)):
                        fo = fop.tile([128, 512], F32)
                        nc.vector.tensor_tensor(fo[:], ps[:],
                                                bo_bc[:, nh * 512:(nh + 1) * 512], ADD)
                        nc.sync.dma_start(out=out[r0:r0 + 128, nh * 512:(nh + 1) * 512],
                                          in_=fo[:])

            for w in range(NW):
                if w == 0:
                    xw = xw0
                else:
                    xw = xwp.tile([128, KQ, 512], MMD, tag="xw")
                    nc.sync.dma_start(out=xw[:], in_=xT[:, w])

                # qT[d, q] = Wq.T @ xw + bq
                qt = qtp.tile([128, H, 512], MMD)
                for m in range(H):
                    ps = ps_m.tile([128, 512], F32, tag="a")
                    for k in range(KQ):
                        nc.tensor.matmul(ps[:], wq_t[:, k, m * 128:(m + 1) * 128],
                                         xw[:, k, :], start=(k == 0), stop=(k == KQ - 1))
                    nc.scalar.activation(out=qt[:, m, :], in_=ps[:], func=Identity,
                                         bias=bq_t[:, m:m + 1], scale=1.0)

                if pending is not None:
                    emit_epilogue(pending)

                at = atp.tile([128, H, 512], MMD)
                z_ps = ps_z.tile([8, 512], F32, tag="z")

                def emit_av(h, ex):
                    # aT[d, q] = v_h.T @ eT (normalized in the deferred epilogue)
                    at_ps = ps_at.tile([128, 512], F32, tag="at")
                    for kc in range(SC):
                        nc.tensor.matmul(at_ps[:], vt[:, kc, h * 128:(h + 1) * 128],
                                         ex[:, kc, :], start=(kc == 0),
                                         stop=(kc == SC - 1))
                    nc.vector.tensor_copy(at[:, h, :], at_ps[:])

                def emit_z(h, es):
                    # softmax denominators for all heads -> one [8,512] PSUM tile
                    for j in range(2):
                        nc.tensor.matmul(z_ps[:], oh_t[:, h, :], es[:, j, :],
                                         start=(h == 0 and j == 0),
                                         stop=(h == H - 1 and j == 1))

                # 2-deep software pipeline: scores+exp for head h, AV of head
                # h-1, Z of head h-2 (Z waits on the gpsimd pair-sum of exps).
                hist = []  # [(h, ex, es)] most recent last
                for h in range(H):
                    pa = ps_sc.tile([128, 2, 512], F32, tag="sc")
                    pb = ps_sc.tile([128, 2, 512], F32, tag="sc")
                    ex = exp_p.tile([128, SC, 512], MMD)
                    for kc in range(SC):
                        phalf = pa if kc < 2 else pb
                        nc.tensor.matmul(phalf[:, kc % 2, :],
                                         kt[:, h, kc * 128:(kc + 1) * 128],
                                         qt[:, h, :], start=True, stop=True)
                    nc.scalar.activation(out=ex[:, 0:2, :], in_=pa[:], func=Exp)
                    nc.scalar.activation(out=ex[:, 2:4, :], in_=pb[:], func=Exp)
                    es = sump.tile([128, 2, 512], MMD)
                    nc.vector.tensor_tensor(es[:], ex[:, 0:2, :], ex[:, 2:4, :], ADD)
                    if hist:
                        emit_av(hist[-1][0], hist[-1][1])
                    if len(hist) >= 2:
                        emit_z(hist[-2][0], hist[-2][2])
                    hist.append((h, ex, es))
                emit_av(hist[-1][0], hist[-1][1])
                emit_z(hist[-2][0], hist[-2][2])
                emit_z(hist[-1][0], hist[-1][2])
                pending = (w, at, z_ps)

            emit_epilogue(pending)
    nc.finalize()
    return nc


def _to_mm(a, mm_dtype):
    if mm_dtype == "bf16":
        import ml_dtypes
        return np.ascontiguousarray(a).astype(ml_dtypes.bfloat16)
    return np.ascontiguousarray(a.astype(np.float32))


def _pack(a, np_, npiece=None):
    """[K, N] -> [128, K//128, N] partition-major (optionally split N into
    npiece contiguous column pieces first: -> [128, npiece, K//128, N//npiece])."""
    K, N = a.shape
    if npiece:
        w = N // npiece
        return np.ascontiguousarray(
            a.reshape(K // 128, 128, npiece, w).transpose(1, 2, 0, 3))
    return np.ascontiguousarray(a.reshape(K // 128, 128, N).transpose(1, 0, 2))


def _prep_inputs(mm_dtype, x, y, Wq, bq, Wk, bk, Wv, bv, Wo, bo):
    x = np.asarray(x, dtype=np.float32)
    y = np.asarray(y, dtype=np.float32).reshape(B, SY, DC)
    scale = 1.0 / math.sqrt(DH)
    Wq_m = _to_mm(_pack(np.asarray(Wq, np.float32), np), mm_dtype)
    wk_m = _to_mm(_pack(np.asarray(Wk, np.float32) * scale, np, npiece=4), mm_dtype)
    wv_m = _to_mm(_pack(np.asarray(Wv, np.float32), np, npiece=2), mm_dtype)
    wo_m = _to_mm(_pack(np.asarray(Wo, np.float32), np), mm_dtype)
    bk_s = np.asarray(bk, dtype=np.float32) * scale
    bo_eff = (np.asarray(bv, dtype=np.float64) @ np.asarray(Wo, dtype=np.float64)
              + np.asarray(bo, dtype=np.float64)).astype(np.float32)
    oh = np.zeros((128, H, H), np.float32)
    for h in range(H):
        oh[:, h, h] = 1.0
    oh = _to_mm(oh.reshape(128, H * H), mm_dtype)
    sel = np.zeros((8, H, 128), np.float32)
    for h in range(H):
        sel[h, h, :] = 1.0
    sel = _to_mm(sel.reshape(8, H * 128), mm_dtype)
    bq = np.asarray(bq, dtype=np.float32)

    in_maps = []
    for c in range(NCORES):
        b, hf = divmod(c, NCORES // B)
        xs = x[b, hf * ROWS:(hf + 1) * ROWS, :]
        # xT [1024, 2048] -> [128, NW, KQ, 512]: window-major pieces so each
        # per-window DMA reads 8KB contiguous per partition.
        xtp = xs.T.reshape(KQ, 128, NW, 512).transpose(1, 2, 0, 3)
        in_maps.append({
            "xT": _to_mm(np.ascontiguousarray(xtp), mm_dtype),
            "yT": _to_mm(_pack(y[b].T, np), mm_dtype),
            "wq": Wq_m, "wk": wk_m, "wv": wv_m, "wo": wo_m,
            "bq": bq, "bk": bk_s, "bo": bo_eff, "oh": oh, "sel": sel,
        })
    return in_maps


def _run(inputs, trace=False, mm_dtype=None):
    _install_shims()
    from concourse.bass_utils import run_bass_kernel_spmd
    mm_dtype = mm_dtype or MM_DTYPE
    if mm_dtype not in _NC_CACHE:
        _NC_CACHE[mm_dtype] = _build_nc(mm_dtype)
    nc = _NC_CACHE[mm_dtype]
    in_maps = _prep_inputs(mm_dtype, **inputs)
    res = run_bass_kernel_spmd(nc, in_maps, list(range(NCORES)), trace=trace)
    outf = np.empty((B, SX, DE), dtype=np.float32)
    for c in range(NCORES):
        b, hf = divmod(c, NCORES // B)
        outf[b, hf * ROWS:(hf + 1) * ROWS, :] = res.results[c]["out"]
    return outf, res


def kernel(**inputs):
    out, _ = _run(inputs, trace=False)
    return out


# revision 10
# speedup vs baseline: 1.0754x; 1.0296x over previous
"""Trainium2 Bass kernel for nn_CrossAttention (B=4, Sx=4096, Sy=512, D=1024, H=8).

Sharding: 8 cores = (batch, query-half). Each core handles 2048 query rows of one
batch; K/V projections for that batch are computed locally (replicated across the
2 cores sharing a batch). The output projection is fully local, so no collectives
are needed; each core writes its own [2048, 1024] output slice.

Layouts are arranged so no on-device transposes are needed:
  qT[d, q]   = Wq.T @ xT        (xT pre-transposed on host)
  kT[d, s]   = (Wk/sqrt(dh)).T @ yT
  v[s, d]    = yT.T @ Wv
  scT[s, q]  = kT_h_chunk.T @ qT_h            (per head, Sy chunks of 128)
  eT         = exp(scT)                        (no max-subtract: |scores| ~ O(1))
  Z[h, q]    = onehot_h.T @ (eT_01 + eT_23)   (gpsimd pre-adds the 4 Sy-chunks
               pairwise so the PE only runs 2 Z-matmuls per head)
  aT[d, q]   = v_chunk.T @ eT, then aT *= (1/Z_h), broadcast across partitions
               via a K=8 row-selector matmul on the [8,512] reciprocal tile
  out[q, n]  = sum_h aT_h_chunk.T @ Wo_h + (bv @ Wo + bo)

v3 pipeline notes:
  - All DRAM inputs are HOST-PACKED partition-major ([128, ...] with large
    contiguous runs per partition) so DMAs move 3-16KB per descriptor instead
    of 0.5-2KB matrix rows (the v2 yt load took 8.6us for 0.75MB).
  - 8 warmup matmuls on a memset tile run during the initial DMA wait so the
    PE p-state is at 2.4GHz when real work starts.
  - exp is fused 1024-wide; scores of head h overlap AV of h-1 and Z of h-2
    (2-deep software pipeline; Z waits on the gpsimd pre-add).
  - PSUM: sc pool 2x[128,2,512] (4 banks), shared qT/bcast/out-proj pool
    2x[128,512] (2), at 1, z 1  -> exactly 8 banks.
  - reciprocal_approx_fast (~5x faster than reciprocal, 18 bits);
    at-copies / bias-adds on DVE (gpsimd = Pool cannot touch PSUM).
All matmuls are bf16 (1 cycle/row on the PE).
"""
import sys
import types
import math
import numpy as np

sys.path.insert(0, "/opt/trn_rl_repo")

B, SX, SY, DE, DC, H, DH = 4, 4096, 512, 1024, 768, 8, 128
NCORES = 8
ROWS = B * SX // NCORES      # 2048 query rows per core
NW = ROWS // 512             # 4 windows of 512 rows
KQ = DE // 128               # 8 k-chunks for q/out projections
KY = DC // 128               # 6 k-chunks for k/v projections
SC = SY // 128               # 4 Sy chunks

MM_DTYPE = "bf16"            # "bf16" | "f32r"  (matmul operand precision)


def _install_shims():
    """antenv.axon_hooks is missing in this image; register the NTFF profile hook
    so trace=True works, and neuter the fish-bucket artifact upload."""
    if "antenv.axon_hooks" in sys.modules:
        return
    import antenv
    mod = types.ModuleType("antenv.axon_hooks")
    _h = [None]
    mod.set_axon_ntff_profile_hook = lambda h: _h.__setitem__(0, h)
    mod.get_axon_ntff_profile_hook = lambda: _h[0]
    sys.modules["antenv.axon_hooks"] = mod
    antenv.axon_hooks = mod
    try:
        from trn_agent_boot.trn_boot import _ntff_profile_via_ctypes
        mod.set_axon_ntff_profile_hook(
            _ntff_profile_via_ctypes("/opt/axon/libaxon_pjrt.so"))
    except Exception:
        pass
    from concourse import bass_utils
    bass_utils.upload_artifacts = lambda tmpdir: "local://" + tmpdir


_NC_CACHE = {}


def _build_nc(mm_dtype):
    from concourse import bacc, mybir
    from concourse.tile import TileContext

    F32 = mybir.dt.float32
    MMD = mybir.dt.bfloat16 if mm_dtype == "bf16" else mybir.dt.float32r
    Identity = mybir.ActivationFunctionType.Identity
    Exp = mybir.ActivationFunctionType.Exp
    ADD = mybir.AluOpType.add

    nc = bacc.Bacc(None, target_bir_lowering=False)
    # all big inputs are host-packed partition-major (see _prep_inputs)
    xT = nc.dram_tensor("xT", [128, NW, KQ, 512], MMD, kind="ExternalInput")
    yT = nc.dram_tensor("yT", [128, KY, SY], MMD, kind="ExternalInput")
    wq = nc.dram_tensor("wq", [128, KQ, DE], MMD, kind="ExternalInput")
    wk = nc.dram_tensor("wk", [128, 4, KY, 256], MMD, kind="ExternalInput")
    wv = nc.dram_tensor("wv", [128, 2, KY, 512], MMD, kind="ExternalInput")
    wo = nc.dram_tensor("wo", [128, KQ, DE], MMD, kind="ExternalInput")
    bq_d = nc.dram_tensor("bq", [DE], F32, kind="ExternalInput")
    bk_d = nc.dram_tensor("bk", [DE], F32, kind="ExternalInput")
    bo_d = nc.dram_tensor("bo", [DE], F32, kind="ExternalInput")
    oh_d = nc.dram_tensor("oh", [128, H * H], MMD, kind="ExternalInput")
    sel_d = nc.dram_tensor("sel", [8, H * 128], MMD, kind="ExternalInput")
    out = nc.dram_tensor("out", [ROWS, DE], F32, kind="ExternalOutput")

    with TileContext(nc) as tc:
        with (
            tc.tile_pool(name="consts", bufs=1) as consts,
            tc.tile_pool(name="xwp", bufs=2) as xwp,
            tc.tile_pool(name="qtp", bufs=2) as qtp,
            tc.tile_pool(name="exp_p", bufs=3) as exp_p,
            tc.tile_pool(name="sump", bufs=3) as sump,
            tc.tile_pool(name="atp", bufs=2) as atp,
            tc.tile_pool(name="fop", bufs=3) as fop,
            tc.tile_pool(name="csp", bufs=2) as csp,
            tc.tile_pool(name="ps_m", bufs=2, space="PSUM") as ps_m,
            tc.tile_pool(name="ps_sc", bufs=2, space="PSUM") as ps_sc,
            tc.tile_pool(name="ps_at", bufs=1, space="PSUM") as ps_at,
            tc.tile_pool(name="ps_z", bufs=1, space="PSUM") as ps_z,
        ):
            # ---- resident constants ----
            wq_t = consts.tile([128, KQ, DE], MMD)
            wo_t = consts.tile([128, KQ, DE], MMD)
            kt = consts.tile([128, H, SY], MMD)       # kT: [d-part, head, Sy]
            vt = consts.tile([128, SC, DE], MMD)      # v: [Sy-part, Sy-chunk, d]
            bo_bc = consts.tile([128, DE], F32)
            bq_t = consts.tile([128, KQ], F32)
            bk_t = consts.tile([128, KQ], F32)
            oh_t = consts.tile([128, H, H], MMD)      # onehot: col h of slice [:,h,:]
            sel_t = consts.tile([8, H, 128], MMD)     # row-selector: row h of [:,h,:]
            yt = consts.tile([128, KY, SY], MMD)
            wkp = consts.tile([128, 4, KY, 256], MMD)  # wk in 4 column pieces
            wvp = consts.tile([128, 2, KY, 512], MMD)  # wv in 2 column halves
            wup = consts.tile([128, 512], MMD)         # warmup scratch

            # PE warmup: ~14 matmuls on a zeroed tile run during the initial DMA
            # wait so the p-state ramp completes before real work arrives.
            nc.vector.memset(wup[:], 0)
            for g in range(2):
                ps = ps_m.tile([128, 512], F32, tag="a")
                for r in range(7):
                    nc.tensor.matmul(ps[:], wup[:, 0:128], wup[:],
                                     start=(r == 0), stop=(r == 6))

            # DMA issue order is execution order per queue. The sync queue is
            # hardware-DGE (fast); gpsimd's is software-DGE (slow) and only
            # carries the tiny bias/selector tensors.
            nc.sync.dma_start(out=yt[:], in_=yT[:])
            nc.gpsimd.dma_start(out=bk_t[:], in_=bk_d[:].rearrange("(m p) -> p m", p=128))
            nc.gpsimd.dma_start(out=bq_t[:], in_=bq_d[:].rearrange("(m p) -> p m", p=128))
            nc.gpsimd.dma_start(out=oh_t[:], in_=oh_d[:].rearrange("p (h m) -> p h m", h=H))
            nc.gpsimd.dma_start(out=sel_t[:], in_=sel_d[:].rearrange("p (h m) -> p h m", h=H))
            nc.gpsimd.dma_start(out=bo_bc[:], in_=bo_d[:].partition_broadcast(128))
            for i in range(4):
                nc.sync.dma_start(out=wkp[:, i], in_=wk[:, i])
            for j in range(2):
                nc.sync.dma_start(out=wvp[:, j], in_=wv[:, j])
            xw0 = xwp.tile([128, KQ, 512], MMD, tag="xw")
            nc.sync.dma_start(out=xw0[:], in_=xT[:, 0])
            nc.sync.dma_start(out=wq_t[:], in_=wq[:])
            nc.sync.dma_start(out=wo_t[:], in_=wo[:])

            # kT[d, s] = (Wk').T @ yT + bk'
            for m in range(8):
                ps = ps_m.tile([128, SY], F32, tag="a")
                for k in range(KY):
                    nc.tensor.matmul(ps[:], wkp[:, m // 2, k, (m % 2) * 128:(m % 2 + 1) * 128],
                                     yt[:, k, :], start=(k == 0), stop=(k == KY - 1))
                nc.scalar.activation(out=kt[:, m, :], in_=ps[:], func=Identity,
                                     bias=bk_t[:, m:m + 1], scale=1.0)
            # v[s, d] = yT.T @ Wv  (bv folded into bo_eff on host)
            for j in range(2):
                for sy in range(SC):
                    ps = ps_m.tile([128, SY], F32, tag="a")
                    for k in range(KY):
                        nc.tensor.matmul(ps[:], yt[:, k, sy * 128:(sy + 1) * 128],
                                         wvp[:, j, k, :], start=(k == 0), stop=(k == KY - 1))
                    nc.vector.tensor_copy(vt[:, sy, j * 512:(j + 1) * 512], ps[:])

            # ---- main loop over query windows of 512 rows ----
            # Window epilogue (normalize + out-proj) is deferred and emitted
            # after the NEXT window's qT matmuls, so the reciprocal/broadcast
            # chain overlaps PE work instead of stalling it.
            pending = None  # (w, at, z_ps) awaiting normalize + out-proj

            def emit_epilogue(p):
                pw, p_at, p_z = p
                zr = csp.tile([8, 512], F32, tag="zr")
                nc.vector.reciprocal_approx_fast(out=zr[:], in_=p_z[:])
                zr_m = csp.tile([8, 512], MMD, tag="zr_m")
                nc.vector.tensor_copy(zr_m[:], zr[:])
                # 6-deep broadcast ring (sc-pool halves + a-pool slots) so the
                # bc matmuls stream back-to-back instead of serializing at the
                # DVE multiply rate.
                bc_ap = [None] * H
                t1 = ps_sc.tile([128, 2, 512], F32, tag="sc")
                bc_ap[0], bc_ap[1] = t1[:, 0, :], t1[:, 1, :]
                t2 = ps_sc.tile([128, 2, 512], F32, tag="sc")
                bc_ap[2], bc_ap[3] = t2[:, 0, :], t2[:, 1, :]
                a1 = ps_m.tile([128, 512], F32, tag="a")
                a2 = ps_m.tile([128, 512], F32, tag="a")
                bc_ap[4], bc_ap[5] = a1[:], a2[:]
                t3 = ps_sc.tile([128, 2, 512], F32, tag="sc")
                bc_ap[6], bc_ap[7] = t3[:, 0, :], t3[:, 1, :]
                for h in range(H):
                    nc.tensor.matmul(bc_ap[h], sel_t[:, h, :], zr_m[:],
                                     start=True, stop=True)
                    nc.vector.tensor_mul(p_at[:, h, :], p_at[:, h, :], bc_ap[h])
                # out[q, n] = sum_h aT_h.T @ Wo_h + bo_eff. Two accumulation
                # groups open at a time, h-major, so each h-block only waits on
                # one normalize multiply and the PE never drains.
                for qc in range(4):
                    psA = ps_m.tile([128, 512], F32, tag="a")
                    psB = ps_m.tile([128, 512], F32, tag="a")
                    for h in range(H):
                        nc.tensor.matmul(psA[:], p_at[:, h, qc * 128:(qc + 1) * 128],
                                         wo_t[:, h, 0:512],
                                         start=(h == 0), stop=(h == H - 1))
                        nc.tensor.matmul(psB[:], p_at[:, h, qc * 128:(qc + 1) * 128],
                                         wo_t[:, h, 512:1024],
                                         start=(h == 0), stop=(h == H - 1))
                    r0 = pw * 512 + qc * 128
                    for nh, ps in ((0, psA), (1, psB)):
                        fo = fop.tile([128, 512], F32)
                        nc.vector.tensor_tensor(fo[:], ps[:],
                                                bo_bc[:, nh * 512:(nh + 1) * 512], ADD)
                        nc.sync.dma_start(out=out[r0:r0 + 128, nh * 512:(nh + 1) * 512],
                                          in_=fo[:])

            for w in range(NW):
                if w == 0:
                    xw = xw0
                else:
                    xw = xwp.tile([128, KQ, 512], MMD, tag="xw")
                    nc.sync.dma_start(out=xw[:], in_=xT[:, w])

                # qT[d, q] = Wq.T @ xw + bq
                qt = qtp.tile([128, H, 512], MMD)
                for m in range(H):
                    ps = ps_m.tile([128, 512], F32, tag="a")
                    for k in range(KQ):
                        nc.tensor.matmul(ps[:], wq_t[:, k, m * 128:(m + 1) * 128],
                                         xw[:, k, :], start=(k == 0), stop=(k == KQ - 1))
                    nc.scalar.activation(out=qt[:, m, :], in_=ps[:], func=Identity,
                                         bias=bq_t[:, m:m + 1], scale=1.0)

                if pending is not None:
                    emit_epilogue(pending)

                at = atp.tile([128, H, 512], MMD)
                z_ps = ps_z.tile([8, 512], F32, tag="z")

                def emit_av(h, ex):
                    # aT[d, q] = v_h.T @ eT (normalized in the deferred epilogue)
                    at_ps = ps_at.tile([128, 512], F32, tag="at")
                    for kc in range(SC):
                        nc.tensor.matmul(at_ps[:], vt[:, kc, h * 128:(h + 1) * 128],
                                         ex[:, kc, :], start=(kc == 0),
                                         stop=(kc == SC - 1))
                    nc.vector.tensor_copy(at[:, h, :], at_ps[:])

                def emit_z(h, es):
                    # softmax denominators for all heads -> one [8,512] PSUM tile
                    nc.tensor.matmul(z_ps[:], oh_t[:, h, :], es[:, 2, :],
                                     start=(h == 0), stop=(h == H - 1))

                # 2-deep software pipeline: scores+exp for head h, AV of head
                # h-1, Z of head h-2 (Z waits on the gpsimd pair-sum of exps).
                hist = []  # [(h, ex, es)] most recent last
                for h in range(H):
                    pa = ps_sc.tile([128, 2, 512], F32, tag="sc")
                    pb = ps_sc.tile([128, 2, 512], F32, tag="sc")
                    ex = exp_p.tile([128, SC, 512], MMD)
                    for kc in range(SC):
                        phalf = pa if kc < 2 else pb
                        nc.tensor.matmul(phalf[:, kc % 2, :],
                                         kt[:, h, kc * 128:(kc + 1) * 128],
                                         qt[:, h, :], start=True, stop=True)
                    nc.scalar.activation(out=ex[:, 0:2, :], in_=pa[:], func=Exp)
                    nc.scalar.activation(out=ex[:, 2:4, :], in_=pb[:], func=Exp)
                    es = sump.tile([128, 3, 512], MMD)
                    nc.vector.tensor_tensor(es[:, 0:2, :], ex[:, 0:2, :], ex[:, 2:4, :], ADD)
                    nc.vector.tensor_tensor(es[:, 2, :], es[:, 0, :], es[:, 1, :], ADD)
                    if hist:
                        emit_av(hist[-1][0], hist[-1][1])
                    if len(hist) >= 2:
                        emit_z(hist[-2][0], hist[-2][2])
                    hist.append((h, ex, es))
                emit_av(hist[-1][0], hist[-1][1])
                emit_z(hist[-2][0], hist[-2][2])
                emit_z(hist[-1][0], hist[-1][2])
                pending = (w, at, z_ps)

            emit_epilogue(pending)
    nc.finalize()
    return nc


def _to_mm(a, mm_dtype):
    if mm_dtype == "bf16":
        import ml_dtypes
        return np.ascontiguousarray(a).astype(ml_dtypes.bfloat16)
    return np.ascontiguousarray(a.astype(np.float32))


def _pack(a, np_, npiece=None):
    """[K, N] -> [128, K//128, N] partition-major (optionally split N into
    npiece contiguous column pieces first: -> [128, npiece, K//128, N//npiece])."""
    K, N = a.shape
    if npiece:
        w = N // npiece
        return np.ascontiguousarray(
            a.reshape(K // 128, 128, npiece, w).transpose(1, 2, 0, 3))
    return np.ascontiguousarray(a.reshape(K // 128, 128, N).transpose(1, 0, 2))


def _prep_inputs(mm_dtype, x, y, Wq, bq, Wk, bk, Wv, bv, Wo, bo):
    x = np.asarray(x, dtype=np.float32)
    y = np.asarray(y, dtype=np.float32).reshape(B, SY, DC)
    scale = 1.0 / math.sqrt(DH)
    Wq_m = _to_mm(_pack(np.asarray(Wq, np.float32), np), mm_dtype)
    wk_m = _to_mm(_pack(np.asarray(Wk, np.float32) * scale, np, npiece=4), mm_dtype)
    wv_m = _to_mm(_pack(np.asarray(Wv, np.float32), np, npiece=2), mm_dtype)
    wo_m = _to_mm(_pack(np.asarray(Wo, np.float32), np), mm_dtype)
    bk_s = np.asarray(bk, dtype=np.float32) * scale
    bo_eff = (np.asarray(bv, dtype=np.float64) @ np.asarray(Wo, dtype=np.float64)
              + np.asarray(bo, dtype=np.float64)).astype(np.float32)
    oh = np.zeros((128, H, H), np.float32)
    for h in range(H):
        oh[:, h, h] = 1.0
    oh = _to_mm(oh.reshape(128, H * H), mm_dtype)
    sel = np.zeros((8, H, 128), np.float32)
    for h in range(H):
        sel[h, h, :] = 1.0
    sel = _to_mm(sel.reshape(8, H * 128), mm_dtype)
    bq = np.asarray(bq, dtype=np.float32)

    in_maps = []
    for c in range(NCORES):
        b, hf = divmod(c, NCORES // B)
        xs = x[b, hf * ROWS:(hf + 1) * ROWS, :]
        # xT [1024, 2048] -> [128, NW, KQ, 512]: window-major pieces so each
        # per-window DMA reads 8KB contiguous per partition.
        xtp = xs.T.reshape(KQ, 128, NW, 512).transpose(1, 2, 0, 3)
        in_maps.append({
            "xT": _to_mm(np.ascontiguousarray(xtp), mm_dtype),
            "yT": _to_mm(_pack(y[b].T, np), mm_dtype),
            "wq": Wq_m, "wk": wk_m, "wv": wv_m, "wo": wo_m,
            "bq": bq, "bk": bk_s, "bo": bo_eff, "oh": oh, "sel": sel,
        })
    return in_maps


def _run(inputs, trace=False, mm_dtype=None):
    _install_shims()
    from concourse.bass_utils import run_bass_kernel_spmd
    mm_dtype = mm_dtype or MM_DTYPE
    if mm_dtype not in _NC_CACHE:
        _NC_CACHE[mm_dtype] = _build_nc(mm_dtype)
    nc = _NC_CACHE[mm_dtype]
    in_maps = _prep_inputs(mm_dtype, **inputs)
    res = run_bass_kernel_spmd(nc, in_maps, list(range(NCORES)), trace=trace)
    outf = np.empty((B, SX, DE), dtype=np.float32)
    for c in range(NCORES):
        b, hf = divmod(c, NCORES // B)
        outf[b, hf * ROWS:(hf + 1) * ROWS, :] = res.results[c]["out"]
    return outf, res


def kernel(**inputs):
    out, _ = _run(inputs, trace=False)
    return out
